# revision 9
# baseline (speedup 1.0000x reference)
"""Trainium2 Bass kernel for nn_AttentionFusion (dense transformer block).

Data-parallel over batch: B=8192 rows sharded as 1024 rows per NeuronCore
across 8 cores; weights replicated. On-chip layout is feature-major:
activations are stored as [128 partitions(features), k_tiles, 1024 rows],
so every matmul is out.T[m,n] = sum_k W.T[k,m] * act.T[k,n] with natural
(host-pre-transposed) weight loads and the contraction on the partition dim.

Algebraic simplifications (validated against the reference to 2e-6):
  - Cross-attention has seq len 1 -> softmax == 1 -> out = v @ wo.T + bo;
    additionally (v @ wv.T) @ wo.T = v @ (wo@wv).T is merged on the host.
  - Self-attention has seq len 2 -> softmax([a,b]) = [sig(a-b), 1-sig(a-b)].
  - LayerNorm / attention-score reductions over features (= partitions) are
    done with small matmuls against ones/head-mask matrices; per-row scalars
    are broadcast back across partitions with rank-1 matmuls.
"""

import numpy as np
import ml_dtypes

import concourse.bacc as bacc
import concourse.mybir as mybir
import concourse.tile as tile
from concourse.bass_utils import run_bass_kernel_spmd

AF = mybir.ActivationFunctionType
ALU = mybir.AluOpType
BF16 = mybir.dt.bfloat16
F32 = mybir.dt.float32

N_CORES = 8
B, IMG_D, TXT_D, H, NH = 8192, 1280, 2048, 1024, 16
HD = H // NH  # 64 head dim
R = B // N_CORES  # 1024 rows per core
P = 128
CH = 2  # row chunks per core
CHS = R // CH  # 512 rows per chunk
KT_I, KT_T, KT_H, KT_F = IMG_D // P, TXT_D // P, H // P, 4 * H // P
EPS = 1e-5

np_bf16 = ml_dtypes.bfloat16


def _chsl(ch):
    return slice(ch * CHS, (ch + 1) * CHS)


def build():
    nc = bacc.Bacc(None, target_bir_lowering=False)

    def din(name, shape, dt=BF16):
        return nc.dram_tensor(name, shape, dt, kind="ExternalInput")

    xiT = din("xiT", [IMG_D, R])
    xtT = din("xtT", [TXT_D, R])
    wiT = din("wiT", [IMG_D, H])
    wtT = din("wtT", [TXT_D, H])
    wvoT = din("wvoT", [H, H])
    wqT = din("wqT", [H, H])
    wkT = din("wkT", [H, H])
    wvT = din("wvT", [H, H])
    woT = din("woT", [H, H])
    w1T = din("w1T", [H, 4 * H])
    w2T = din("w2T", [4 * H, H])
    gwiT = din("gwiT", [H, H])
    gwtT = din("gwtT", [H, H])

    bias_names = ["bi", "bt", "bvo", "sbq", "sbk", "sbv", "sbo", "fb2", "gb",
                  "n1g", "n1b", "n2g", "n2b", "n3g", "n3b"]
    bias_d = {n: din(n, [KT_H, P], F32) for n in bias_names}
    fb1_d = din("fb1", [KT_F, P], F32)
    hmask_d = din("hmask", [P, 2])
    hmaskT_d = din("hmaskT", [2, P])

    # DRAM spill for imgp/txtp between P3 and P8 (frees SBUF during SA/FFN)
    imgp_d = nc.dram_tensor("imgp_spill", [P, KT_H, R], BF16)
    txtp_d = nc.dram_tensor("txtp_spill", [P, KT_H, R], BF16)

    outT = nc.dram_tensor("outT", [H, R], F32, kind="ExternalOutput")

    with tile.TileContext(nc) as tc:
        def open_pool(**kw):
            cm = tc.tile_pool(**kw)
            return cm, cm.__enter__()

        # -------- constants (whole kernel) --------
        const_cm, const = open_pool(name="const", bufs=1)
        ones_col = const.tile([P, 1], BF16)
        nc.vector.memset(ones_col[:], 1.0)
        ones_row = const.tile([1, P], F32)
        nc.vector.memset(ones_row[:], 1.0)
        eps_t = const.tile([1, 1], F32)
        nc.vector.memset(eps_t[:], EPS)
        zero_col = const.tile([P, 1], F32)
        nc.vector.memset(zero_col[:], 0.0)
        bias_sb = {}
        for n in bias_names:
            t = const.tile([P, KT_H], F32, tag=f"b_{n}", name=f"b_{n}")
            nc.sync.dma_start(t[:], bias_d[n].rearrange("t p -> p t"))
            bias_sb[n] = t
        fb1_sb = const.tile([P, KT_F], F32, tag="b_fb1")
        nc.sync.dma_start(fb1_sb[:], fb1_d.rearrange("t p -> p t"))
        hmask_sb = const.tile([P, 2], BF16)
        nc.sync.dma_start(hmask_sb[:], hmask_d[:, :])
        hmaskT_sb = const.tile([2, P], BF16)
        nc.sync.dma_start(hmaskT_sb[:], hmaskT_d[:, :])

        # -------- shared SBUF pools (whole kernel) --------
        wpool_cm, wpool = open_pool(name="wpool", bufs=2)   # slot 8KB -> 16KB/part
        tpool_cm, tpool = open_pool(name="tpool", bufs=6)   # slot 2KB -> 12KB/part
        spool_cm, spool = open_pool(name="spool", bufs=4)   # slot 2KB -> 8KB/part
        # long-lived activation pool: six recycled bf16 slots + one f32 slot
        acts_cm, acts = open_pool(name="acts", bufs=1)      # 6*16 + 32 = 128KB/part

        def act_tile(tag, name, dt=BF16):
            return acts.tile([P, KT_H, R], dt, tag=tag, name=name)

        pmain = None
        paux = None

        def load_w(wT_d, kt, mt, name):
            """Weight slice wT[:, mt*128:(mt+1)*128] as [128, kt, 128] (bf16)."""
            t = wpool.tile([P, KT_F, P], BF16, tag="w", name=name)
            nc.sync.dma_start(
                t[:, :kt, :],
                wT_d[:, mt * P:(mt + 1) * P].rearrange("(k p) m -> p k m", p=P),
            )
            return t

        def mm_layer(srcs, mt, evict, wname="w"):
            """Accumulate over all (w_d, act, kt) in srcs; evict(mt, ch, ps)."""
            wts = [load_w(w_d, kt, mt, f"{wname}{i}") for i, (w_d, _, kt) in enumerate(srcs)]
            nk_tot = sum(kt for (_, _, kt) in srcs)
            for ch in range(CH):
                ps = pmain.tile([P, CHS], F32, tag="mm", name="ps_mm")
                i = 0
                for (w_d, act, kt), wt in zip(srcs, wts):
                    for k in range(kt):
                        nc.tensor.matmul(
                            ps[:], lhsT=wt[:, k, :], rhs=act[:, k, _chsl(ch)],
                            start=(i == 0), stop=(i == nk_tot - 1),
                        )
                        i += 1
                evict(mt, ch, ps)

        def evict_bias(dst, bname):
            b = bias_sb[bname]

            def _e(mt, ch, ps):
                nc.scalar.activation(
                    dst[:, mt, _chsl(ch)], ps[:], AF.Identity,
                    bias=b[:, mt:mt + 1], scale=1.0,
                )
            return _e

        def evict_bias_res(dst, bname, res):
            b = bias_sb[bname]

            def _e(mt, ch, ps):
                nc.vector.scalar_tensor_tensor(
                    dst[:, mt, _chsl(ch)], ps[:], b[:, mt:mt + 1],
                    res[:, mt, _chsl(ch)], op0=ALU.add, op1=ALU.add,
                )
            return _e

        def layernorm(x_bf, g_name, b_name, out_bf):
            """out = LN(x) over the feature (= partition) dim, feature-major."""
            g = bias_sb[g_name]
            bb = bias_sb[b_name]
            for ch in range(CH):
                st_s = paux.tile([1, CHS], F32, tag="stat_s", name="ln_st_s")
                for k in range(KT_H):
                    nc.tensor.matmul(st_s[:], lhsT=ones_col[:],
                                     rhs=x_bf[:, k, _chsl(ch)],
                                     start=(k == 0), stop=(k == KT_H - 1))
                st_q = paux.tile([1, CHS], F32, tag="stat_q", name="ln_st_q")
                for k in range(KT_H):
                    x2 = tpool.tile([P, CHS], BF16, tag="tmp", name="ln_x2")
                    nc.vector.tensor_mul(out=x2[:], in0=x_bf[:, k, _chsl(ch)],
                                         in1=x_bf[:, k, _chsl(ch)])
                    nc.tensor.matmul(st_q[:], lhsT=ones_col[:], rhs=x2[:],
                                     start=(k == 0), stop=(k == KT_H - 1))
                m_sb = spool.tile([1, CHS], F32, tag="small", name="ln_m")
                nc.vector.tensor_scalar_mul(m_sb[:], st_s[:], 1.0 / H)
                msq = spool.tile([1, CHS], F32, tag="small", name="ln_msq")
                nc.vector.tensor_mul(out=msq[:], in0=m_sb[:], in1=m_sb[:])
                var = spool.tile([1, CHS], F32, tag="small", name="ln_var")
                nc.vector.scalar_tensor_tensor(var[:], st_q[:], 1.0 / H, msq[:],
                                               op0=ALU.mult, op1=ALU.subtract)
                sd = spool.tile([1, CHS], F32, tag="small", name="ln_sd")
                nc.scalar.activation(sd[:], var[:], AF.Sqrt, bias=eps_t[:], scale=1.0)
                inv = spool.tile([1, CHS], F32, tag="small", name="ln_inv")
                nc.vector.reciprocal(inv[:], sd[:])
                mb = paux.tile([P, CHS], F32, tag="bcm", name="ln_mb")
                nc.tensor.matmul(mb[:], lhsT=ones_row[:], rhs=m_sb[:],
                                 start=True, stop=True)
                ib = paux.tile([P, CHS], F32, tag="bci", name="ln_ib")
                nc.tensor.matmul(ib[:], lhsT=ones_row[:], rhs=inv[:],
                                 start=True, stop=True)
                for k in range(KT_H):
                    t1 = tpool.tile([P, CHS], F32, tag="tmp", name="ln_t1")
                    nc.vector.tensor_sub(out=t1[:], in0=x_bf[:, k, _chsl(ch)], in1=mb[:])
                    t2 = tpool.tile([P, CHS], F32, tag="tmp", name="ln_t2")
                    nc.vector.tensor_mul(out=t2[:], in0=t1[:], in1=ib[:])
                    nc.scalar.activation(out_bf[:, k, _chsl(ch)], t2[:], AF.Identity,
                                         bias=bb[:, k:k + 1], scale=g[:, k:k + 1])

        # ================= P0/P1: input projections (streamed) =============
        imgp = act_tile("S1", "imgp")
        txtp = act_tile("S2", "txtp")

        def input_proj(xT_d, w_d, kt_in, bname, dst):
            for ch in range(CH):
                pss = [pmain.tile([P, CHS], F32, tag=f"mm{mt}", name=f"ps{mt}")
                       for mt in range(KT_H)]
                for k in range(kt_in):
                    wt = wpool.tile([P, H], BF16, tag="w", name="wrow")
                    nc.sync.dma_start(wt[:], w_d[k * P:(k + 1) * P, :])
                    xs = tpool.tile([P, CHS], BF16, tag="tmp", name="xslice")
                    nc.sync.dma_start(xs[:], xT_d[k * P:(k + 1) * P, _chsl(ch)])
                    for mt in range(KT_H):
                        nc.tensor.matmul(pss[mt][:], lhsT=wt[:, mt * P:(mt + 1) * P],
                                         rhs=xs[:], start=(k == 0), stop=(k == kt_in - 1))
                for mt in range(KT_H):
                    nc.scalar.activation(dst[:, mt, _chsl(ch)], pss[mt][:], AF.Identity,
                                         bias=bias_sb[bname][:, mt:mt + 1], scale=1.0)

        with tc.tile_pool(name="pmm01", bufs=1, space="PSUM") as pmain:
            input_proj(xiT, wiT, KT_I, "bi", imgp)
            input_proj(xtT, wtT, KT_T, "bt", txtp)
        nc.sync.dma_start(imgp_d[:, :, :], imgp[:])
        nc.sync.dma_start(txtp_d[:, :, :], txtp[:])

        # ============ P2/P3: merged cross-attention + LN ============
        c0 = act_tile("S3", "c0")
        c1 = act_tile("S4", "c1")

        with (
            tc.tile_pool(name="pmm23", bufs=3, space="PSUM") as pmain,
            tc.tile_pool(name="paux23", bufs=1, space="PSUM") as paux,
            tc.tile_pool(name="pca", bufs=1) as pca,
        ):
            x0 = pca.tile([P, KT_H, R], BF16, tag="x", name="x0")
            for mt in range(KT_H):
                mm_layer([(wvoT, txtp, KT_H)], mt, evict_bias_res(x0, "bvo", imgp),
                         wname="wvo")
            layernorm(x0, "n1g", "n1b", c0)
            x1 = pca.tile([P, KT_H, R], BF16, tag="x", name="x1")
            for mt in range(KT_H):
                mm_layer([(wvoT, imgp, KT_H)], mt, evict_bias_res(x1, "bvo", txtp),
                         wname="wvo")
            layernorm(x1, "n2g", "n2b", c1)

        # ================ P4: self-attention (seq len 2) ================
        o0 = act_tile("S5", "o0")
        o1 = act_tile("S6", "o1")
        v0 = act_tile("S1", "v0")  # reuses imgp slot (spilled to DRAM)
        v1 = act_tile("S2", "v1")  # reuses txtp slot

        with (
            tc.tile_pool(name="pmm4", bufs=2, space="PSUM") as pmain,
            tc.tile_pool(name="pd", bufs=4, space="PSUM") as pd,
            tc.tile_pool(name="pab", bufs=1, space="PSUM") as pab,
            tc.tile_pool(name="pqk", bufs=1) as pqk,
        ):
            def qkv(w_d, act, bname, mt, dst_t, dst_mt=None, wname="wq"):
                wt = load_w(w_d, KT_H, mt, wname)
                for ch in range(CH):
                    ps = pmain.tile([P, CHS], F32, tag="mm", name="ps_qkv")
                    for k in range(KT_H):
                        nc.tensor.matmul(ps[:], lhsT=wt[:, k, :],
                                         rhs=act[:, k, _chsl(ch)],
                                         start=(k == 0), stop=(k == KT_H - 1))
                    b = bias_sb[bname]
                    if dst_mt is None:
                        nc.scalar.activation(dst_t[:, _chsl(ch)], ps[:], AF.Identity,
                                             bias=b[:, mt:mt + 1], scale=1.0)
                    else:
                        nc.scalar.activation(dst_t[:, dst_mt, _chsl(ch)], ps[:],
                                             AF.Identity, bias=b[:, mt:mt + 1], scale=1.0)

            for mt in range(KT_H):
                q0t = pqk.tile([P, R], BF16, tag="q0t")
                q1t = pqk.tile([P, R], BF16, tag="q1t")
                k0t = pqk.tile([P, R], BF16, tag="k0t")
                k1t = pqk.tile([P, R], BF16, tag="k1t")
                qkv(wqT, c0, "sbq", mt, q0t, wname="wq")
                qkv(wqT, c1, "sbq", mt, q1t, wname="wq")
                qkv(wkT, c0, "sbk", mt, k0t, wname="wk")
                qkv(wkT, c1, "sbk", mt, k1t, wname="wk")
                qkv(wvT, c0, "sbv", mt, v0, dst_mt=mt, wname="wv")
                qkv(wvT, c1, "sbv", mt, v1, dst_mt=mt, wname="wv")
                # s_i0 - s_i1 = q_i . (k0 - k1) per head -> sigmoid -> attn weight
                kd = pqk.tile([P, R], BF16, tag="kd")
                nc.vector.tensor_sub(out=kd[:], in0=k0t[:], in1=k1t[:])
                m0 = pqk.tile([P, R], BF16, tag="m0")
                nc.vector.tensor_mul(out=m0[:], in0=q0t[:], in1=kd[:])
                m1 = pqk.tile([P, R], BF16, tag="m1")
                nc.vector.tensor_mul(out=m1[:], in0=q1t[:], in1=kd[:])
                hm2 = hmask_sb[:, :]    # [128, 2] local-head one-hot
                hmT2 = hmaskT_sb[:, :]  # [2, 128]
                for ch in range(CH):
                    diff = tpool.tile([P, CHS], BF16, tag="tmp", name="att_diff")
                    nc.vector.tensor_sub(out=diff[:], in0=v0[:, mt, _chsl(ch)],
                                         in1=v1[:, mt, _chsl(ch)])
                    for m_t, o_t, nm in ((m0, o0, "A"), (m1, o1, "B")):
                        dmm = pd.tile([2, CHS], F32, tag="dmm", name=f"dmm{nm}")
                        nc.tensor.matmul(dmm[:], lhsT=hm2, rhs=m_t[:, _chsl(ch)],
                                         start=True, stop=True)
                        a_t = spool.tile([2, CHS], BF16, tag="small", name=f"a{nm}")
                        nc.scalar.activation(a_t[:], dmm[:], AF.Sigmoid,
                                             bias=zero_col[0:2, :],
                                             scale=float(1.0 / np.sqrt(HD)))
                        ab = pab.tile([P, CHS], F32, tag=f"ab{nm}", name=f"ab{nm}")
                        nc.tensor.matmul(ab[:], lhsT=hmT2, rhs=a_t[:],
                                         start=True, stop=True)
                        t_t = tpool.tile([P, CHS], BF16, tag="tmp", name=f"att_t{nm}")
                        nc.vector.tensor_mul(out=t_t[:], in0=diff[:], in1=ab[:])
                        nc.vector.tensor_add(out=o_t[:, mt, _chsl(ch)], in0=t_t[:],
                                             in1=v1[:, mt, _chsl(ch)])

        # ========= P5: self-attention out-proj + residual + LN3 =========
        r0 = act_tile("S1", "r0")  # reuses v0 slot
        r1 = act_tile("S2", "r1")  # reuses v1 slot
        with (
            tc.tile_pool(name="pmm5", bufs=3, space="PSUM") as pmain,
            tc.tile_pool(name="paux5", bufs=1, space="PSUM") as paux,
            tc.tile_pool(name="psa", bufs=1) as psa,
        ):
            y0 = psa.tile([P, KT_H, R], BF16, tag="y", name="y0")
            for mt in range(KT_H):
                mm_layer([(woT, o0, KT_H)], mt, evict_bias_res(y0, "sbo", c0), wname="wo")
            layernorm(y0, "n3g", "n3b", r0)
            y1 = psa.tile([P, KT_H, R], BF16, tag="y", name="y1")
            for mt in range(KT_H):
                mm_layer([(woT, o1, KT_H)], mt, evict_bias_res(y1, "sbo", c1), wname="wo")
            layernorm(y1, "n3g", "n3b", r1)

        # ===== P6/P7: FFN both positions; pooled = r0+f0+r1+f1 (f32) =====
        pooled = acts.tile([P, KT_H, R], F32, tag="SP", name="pooled")
        with (
            tc.tile_pool(name="pmm67", bufs=4, space="PSUM") as pmain,
            tc.tile_pool(name="pffn", bufs=1) as pffn,
        ):
            for pos, (r_act, first) in enumerate([(r0, True), (r1, False)]):
                for ch in range(CH):
                    h = pffn.tile([P, KT_F, CHS], BF16, tag="h", name="h")
                    for mt in range(KT_F):
                        wt = load_w(w1T, KT_H, mt, "w1")
                        ps = pmain.tile([P, CHS], F32, tag="mm", name="ps_f1")
                        for k in range(KT_H):
                            nc.tensor.matmul(ps[:], lhsT=wt[:, k, :],
                                             rhs=r_act[:, k, _chsl(ch)],
                                             start=(k == 0), stop=(k == KT_H - 1))
                        nc.scalar.activation(h[:, mt, :], ps[:], AF.Gelu,
                                             bias=fb1_sb[:, mt:mt + 1], scale=1.0)
                    for mt in range(KT_H):
                        wt = load_w(w2T, KT_F, mt, "w2")
                        ps = pmain.tile([P, CHS], F32, tag="mm", name="ps_f2")
                        for k in range(KT_F):
                            nc.tensor.matmul(ps[:], lhsT=wt[:, k, :],
                                             rhs=h[:, k, :],
                                             start=(k == 0), stop=(k == KT_F - 1))
                        if first:
                            nc.vector.scalar_tensor_tensor(
                                pooled[:, mt, _chsl(ch)], ps[:],
                                bias_sb["fb2"][:, mt:mt + 1],
                                r_act[:, mt, _chsl(ch)], op0=ALU.add, op1=ALU.add)
                        else:
                            tmp = tpool.tile([P, CHS], F32, tag="tmp", name="ffn_tmp")
                            nc.vector.scalar_tensor_tensor(
                                tmp[:], ps[:], bias_sb["fb2"][:, mt:mt + 1],
                                r_act[:, mt, _chsl(ch)], op0=ALU.add, op1=ALU.add)
                            nc.vector.tensor_add(out=pooled[:, mt, _chsl(ch)],
                                                 in0=pooled[:, mt, _chsl(ch)], in1=tmp[:])

        # ================= P8: gate + final combine =================
        imgp2 = act_tile("S3", "imgp2")  # reuses c0 slot
        nc.sync.dma_start(imgp2[:], imgp_d[:, :, :])
        txtp2 = act_tile("S4", "txtp2")  # reuses c1 slot
        nc.sync.dma_start(txtp2[:], txtp_d[:, :, :])
        with (
            tc.tile_pool(name="pmm8", bufs=4, space="PSUM") as pmain,
            tc.tile_pool(name="pg", bufs=2) as pg,
        ):
            for mt in range(KT_H):
                wgi = load_w(gwiT, KT_H, mt, "wgi")
                wgt = load_w(gwtT, KT_H, mt, "wgt")
                for ch in range(CH):
                    ps = pmain.tile([P, CHS], F32, tag="mm", name="ps_g")
                    for k in range(KT_H):
                        nc.tensor.matmul(ps[:], lhsT=wgi[:, k, :],
                                         rhs=imgp2[:, k, _chsl(ch)],
                                         start=(k == 0), stop=False)
                    for k in range(KT_H):
                        nc.tensor.matmul(ps[:], lhsT=wgt[:, k, :],
                                         rhs=txtp2[:, k, _chsl(ch)],
                                         start=False, stop=(k == KT_H - 1))
                    g = pg.tile([P, CHS], BF16, tag="g")
                    nc.scalar.activation(g[:], ps[:], AF.Sigmoid,
                                         bias=bias_sb["gb"][:, mt:mt + 1], scale=1.0)
                    diff = pg.tile([P, CHS], BF16, tag="gdiff")
                    nc.vector.tensor_sub(out=diff[:], in0=imgp2[:, mt, _chsl(ch)],
                                         in1=txtp2[:, mt, _chsl(ch)])
                    t = pg.tile([P, CHS], F32, tag="gt")
                    nc.vector.tensor_mul(out=t[:], in0=g[:], in1=diff[:])
                    u = pg.tile([P, CHS], F32, tag="gu")
                    nc.vector.scalar_tensor_tensor(u[:], pooled[:, mt, _chsl(ch)], 0.5,
                                                   t[:], op0=ALU.mult, op1=ALU.add)
                    fin = pg.tile([P, CHS], F32, tag="gfin")
                    nc.vector.tensor_add(out=fin[:], in0=u[:], in1=txtp2[:, mt, _chsl(ch)])
                    nc.sync.dma_start(outT[mt * P:(mt + 1) * P, _chsl(ch)], fin[:])

        acts_cm.__exit__(None, None, None)
        spool_cm.__exit__(None, None, None)
        tpool_cm.__exit__(None, None, None)
        wpool_cm.__exit__(None, None, None)
        const_cm.__exit__(None, None, None)

    nc.compile()
    return nc


def host_prep(inputs):
    """Host-side preprocessing: merge CA weights, transpose, cast, shard."""
    f = {k: np.asarray(v, dtype=np.float32) for k, v in inputs.items()}

    def bf(x):
        return np.ascontiguousarray(x).astype(np_bf16)

    def bias128(x, kt):
        return np.ascontiguousarray(np.asarray(x, np.float32).reshape(kt, P))

    ca_wv = np.split(f["ca_wqkv"], 3, axis=0)[2]
    ca_bv = f["ca_bqkv"][2 * H:]
    w_vo = f["ca_wo"] @ ca_wv
    b_vo = f["ca_wo"] @ ca_bv + f["ca_bo"]

    sa_wq, sa_wk, sa_wv = np.split(f["sa_wqkv"], 3, axis=0)
    sa_bq, sa_bk, sa_bv = np.split(f["sa_bqkv"], 3)

    gwi = f["gate_w"][:, :H]
    gwt = f["gate_w"][:, H:]

    lh = np.arange(P) // HD  # local head index within a 128-feature tile
    hmask = np.ascontiguousarray((lh[:, None] == np.arange(2)[None, :]).astype(np_bf16))
    hmaskT = np.ascontiguousarray(hmask.T)

    shared = {
        "wiT": bf(f["Wi"].T), "wtT": bf(f["Wt"].T),
        "wvoT": bf(w_vo.T),
        "wqT": bf(sa_wq.T), "wkT": bf(sa_wk.T), "wvT": bf(sa_wv.T),
        "woT": bf(f["sa_wo"].T),
        "w1T": bf(f["ffn_w1"].T), "w2T": bf(f["ffn_w2"].T),
        "gwiT": bf(gwi.T), "gwtT": bf(gwt.T),
        "bi": bias128(f["bi"], KT_H), "bt": bias128(f["bt"], KT_H),
        "bvo": bias128(b_vo, KT_H),
        "sbq": bias128(sa_bq, KT_H), "sbk": bias128(sa_bk, KT_H),
        "sbv": bias128(sa_bv, KT_H), "sbo": bias128(f["sa_bo"], KT_H),
        "fb1": bias128(f["ffn_b1"], KT_F), "fb2": bias128(f["ffn_b2"], KT_H),
        "gb": bias128(f["gate_b"], KT_H),
        "n1g": bias128(f["n1_g"], KT_H), "n1b": bias128(f["n1_b"], KT_H),
        "n2g": bias128(f["n2_g"], KT_H), "n2b": bias128(f["n2_b"], KT_H),
        "n3g": bias128(f["n3_g"], KT_H), "n3b": bias128(f["n3_b"], KT_H),
        "hmask": np.ascontiguousarray(hmask), "hmaskT": hmaskT,
    }

    xiT = f["image_features"].T.astype(np_bf16)  # [IMG_D, B]
    xtT = f["text_features"].T.astype(np_bf16)
    in_maps = []
    for c in range(N_CORES):
        m = dict(shared)
        m["xiT"] = np.ascontiguousarray(xiT[:, c * R:(c + 1) * R])
        m["xtT"] = np.ascontiguousarray(xtT[:, c * R:(c + 1) * R])
        in_maps.append(m)
    return in_maps


_NC_CACHE = None


def kernel(**inputs) -> np.ndarray:
    global _NC_CACHE
    if _NC_CACHE is None:
        _NC_CACHE = build()
    nc = _NC_CACHE
    in_maps = host_prep(inputs)
    res = run_bass_kernel_spmd(nc, in_maps, core_ids=list(range(N_CORES)))
    out = np.empty((B, H), np.float32)
    for c in range(N_CORES):
        out[c * R:(c + 1) * R, :] = res.results[c]["outT"].T
    return out


if __name__ == "__main__":
    nc = build()
    print("built OK")
